# revision 39
# baseline (speedup 1.0000x reference)
"""Trainium2 Bass kernel for nn_AttentionPool (segment softmax-pool over gene/spot edges).

Math: out[g] = (sum_{s in S_g} e_s * emb[s]) / (sum_{s in S_g} e_s),
      e_s = exp(logit_s - c),  logit = tanh(emb @ W.T + b) @ v
where S_g is the *set* of distinct spots expressing gene g (duplicate edges
count once), and empty genes produce 0. Any shift c cancels in the ratio;
c = 5.0 (> max logit ~3.96 for this problem's xavier init) keeps every e_s
in fp8e4m3 range with wide margin on both ends.

Sharding: 2500 genes per core x 8 cores (padded to 2560 = 20 tiles of 128).
Host marshals the edge list into each core's dense fp8 {0,1} mask slab in the
DoubleRowSwInterleave weight layout (pair-interleaved ktiles, gene columns
reversed) so each [128,2,128] chunk is a K=256 matmul lhsT at 0.5 cycles/row.
X = [e*emb | e] is carried as fp8 hi plus 64x-scaled lo residual, merged as
258 rhs columns per chunk so one LDWEIGHTS serves both; the two PSUM column
blocks are recombined as hi + lo/64, giving bf16-level accuracy at 2x rate.
Main loop runs 4 gene tiles per PSUM generation so the tensor engine can
interleave across tiles while X groups are still being produced.
"""

import sys

sys.path.insert(0, "/opt/trn_rl_repo")

import numpy as np
import ml_dtypes

import concourse.mybir as mybir
import concourse.tile as tile
from concourse import bacc
from concourse.bass import ts
from concourse.bass_utils import run_bass_kernel_spmd
from concourse.bass_interp import get_hw_module

F32 = mybir.dt.float32
BF16 = mybir.dt.bfloat16
F8 = mybir.dt.float8e4

N_SPOTS = 4096
N_GENES = 20000
D = 128
N_CORES = 8
G_PER = N_GENES // N_CORES  # 2500
P = 128
KCH = N_SPOTS // P  # 32 spot chunks of 128
KK = KCH // 2  # 16 double-chunks of 256 for DoubleRow
NX = D + 1  # X columns: [e*emb | e]
LO_SCALE = 64.0  # keeps fp8 lo residuals out of the subnormal range
C_SHIFT = 5.0  # logit shift; exact value cancels in the num/den ratio
NG = 8  # X built in 8 groups of 4 chunks
GS = KCH // NG  # 4
TG = 4  # gene tiles per PSUM generation


def build_nc(T):
    """Build the single-core Bass program (SPMD across 8 cores).

    T = number of 128-gene tiles per core (20 for the real problem).
    """
    nc = bacc.Bacc("TRN2", target_bir_lowering=False, debug=False, num_devices=N_CORES)

    maskf = nc.dram_tensor("maskf", [T // 2, P, 2 * KCH * P], F8, kind="ExternalInput")
    embT = nc.dram_tensor("embT", [P, N_SPOTS], BF16, kind="ExternalInput")
    embc = nc.dram_tensor("embc", [P, KCH * D], BF16, kind="ExternalInput")
    wt = nc.dram_tensor("wt", [D, D], BF16, kind="ExternalInput")
    bb = nc.dram_tensor("bb", [D, 1], F32, kind="ExternalInput")
    vv = nc.dram_tensor("vv", [D, 1], BF16, kind="ExternalInput")
    out = nc.dram_tensor("out", [T // TG, P, TG * D], F32, kind="ExternalOutput")

    with tile.TileContext(nc) as tc:
        with (
            tc.tile_pool(name="const", bufs=1) as constp,
            tc.tile_pool(name="xfp", bufs=1) as xfp,
            tc.tile_pool(name="maskp", bufs=T // 2) as maskp,
            tc.tile_pool(name="outp", bufs=3) as outp,
            tc.tile_pool(name="php", bufs=2, space="PSUM") as php,
            tc.tile_pool(name="pep", bufs=1, space="PSUM") as pep,
            tc.tile_pool(name="ptp", bufs=TG + 1, space="PSUM") as ptp,
        ):
            # ---- constants into SBUF (sync ring; scalar/ACT stays clean) ----
            wt_sb = constp.tile([P, D], BF16)
            nc.sync.dma_start(out=wt_sb[:], in_=wt[:])
            b_sb = constp.tile([P, 1], F32)
            nc.sync.dma_start(out=b_sb[:], in_=bb[:])
            v_sb = constp.tile([P, 1], BF16)
            nc.sync.dma_start(out=v_sb[:], in_=vv[:])
            HS = N_SPOTS // 2
            embT_a = constp.tile([P, HS], BF16)
            embT_b = constp.tile([P, HS], BF16)
            nc.gpsimd.dma_start(out=embT_a[:], in_=embT[:, 0:HS])
            nc.sync.dma_start(out=embT_b[:], in_=embT[:, HS:])
            # embc triggers on the scalar ring, ahead of any ACT compute
            HC = KCH * D // 2
            embc_a = constp.tile([P, HC], BF16)
            embc_b = constp.tile([P, HC], BF16)
            nc.gpsimd.dma_start(out=embc_a[:], in_=embc[:, 0:HC])
            nc.scalar.dma_start(out=embc_b[:], in_=embc[:, HC:])

            def embT_cols(lo, width):
                if lo < HS:
                    return embT_a[:, lo : lo + width]
                return embT_b[:, lo - HS : lo - HS + width]

            negc_sb = constp.tile([P, 1], F32)
            nc.vector.memset(negc_sb[:], -C_SHIFT)

            th_sb = constp.tile([P, N_SPOTS], BF16)  # tanh(W h + b).T  [j, s]
            e_sb = constp.tile([P, KCH], F32)  # e in spot-partition layout
            # per-group X tiles: [Xhi | Xlo] merged per chunk, fp8
            xmg = [
                constp.tile([P, GS * 2 * NX], F8, name=f"xmg{g}") for g in range(NG)
            ]

            # ---- mask slab DMAs, two tiles per transfer, issued up front ----
            mts = []
            for u in range(T // 2):
                mt2 = maskp.tile([P, 2 * KCH * P], F8, name=f"mt{u}", tag="mt")
                nc.gpsimd.dma_start(out=mt2[:], in_=maskf[u])
                mts.append(mt2[:, 0 : KCH * P])
                mts.append(mt2[:, KCH * P : 2 * KCH * P])

            # ---- prologue: th = tanh(W@emb.T + b), logits, e ----
            NCH = N_SPOTS // 512  # 8 th chunks of 512 spots
            pe = pep.tile([P, KCH], F32)
            for c in range(NCH):
                ph = php.tile([P, 512], F32, tag="ph")
                nc.tensor.matmul(
                    out=ph[:], lhsT=wt_sb[:], rhs=embT_cols(c * 512, 512),
                    start=True, stop=True,
                )
                nc.scalar.activation(
                    out=th_sb[:, ts(c, 512)], in_=ph[:],
                    func=mybir.ActivationFunctionType.Tanh, bias=b_sb[:, 0:1],
                )
                # logits in spot-partition layout: [128 s, 1] = th_k.T @ v
                for k in range(4 * c, 4 * c + 4):
                    nc.tensor.matmul(
                        out=pe[:, k : k + 1], lhsT=th_sb[:, ts(k, P)], rhs=v_sb[:],
                        start=True, stop=True,
                    )
                nc.scalar.activation(
                    out=e_sb[:, 4 * c : 4 * c + 4], in_=pe[:, 4 * c : 4 * c + 4],
                    func=mybir.ActivationFunctionType.Exp, bias=negc_sb[:, 0:1],
                )

            # ---- X = [e*emb | e] as fp8 hi + 64x lo, 8 groups of 4 chunks ----
            # Pool: mul + lo-cast; ACT: hi-cast; DVE: sub
            xf = xfp.tile([P, KCH * NX], F32)
            xd = xfp.tile([P, KCH * NX], F32)
            xf3 = xf[:].rearrange("p (k n) -> p k n", n=NX)
            xd3 = xd[:].rearrange("p (k n) -> p k n", n=NX)
            emb3a = embc_a[:].rearrange("p (k d) -> p k d", d=D)
            emb3b = embc_b[:].rearrange("p (k d) -> p k d", d=D)
            e3 = e_sb[:].rearrange("p k -> p k ()")
            for g in range(NG):
                ks = slice(g * GS, (g + 1) * GS)
                if g < NG // 2:
                    embsrc = emb3a[:, ks, :]
                else:
                    embsrc = emb3b[:, slice(g * GS - KCH // 2, (g + 1) * GS - KCH // 2), :]
                ebc = e3[:, ks, :].to_broadcast([P, GS, D])
                xg3 = xmg[g][:].rearrange("p (c n) -> p c n", n=2 * NX)
                hi3 = xg3[:, :, 0:NX]
                lo3 = xg3[:, :, NX : 2 * NX]
                nc.vector.tensor_mul(out=xf3[:, ks, 0:D], in0=embsrc, in1=ebc)
                nc.vector.tensor_copy(out=xf3[:, ks, D : D + 1], in_=e3[:, ks, :])
                nc.scalar.activation(
                    out=hi3, in_=xf3[:, ks, :], func=mybir.ActivationFunctionType.Copy
                )
                nc.vector.tensor_sub(out=xd3[:, ks, :], in0=xf3[:, ks, :], in1=hi3)
                nc.scalar.activation(
                    out=lo3, in_=xd3[:, ks, :],
                    func=mybir.ActivationFunctionType.Copy, scale=LO_SCALE,
                )

            # ---- main loop: TG gene tiles per PSUM generation ----
            for tg in range(T // TG):
                tls = list(range(tg * TG, (tg + 1) * TG))
                pts = [
                    ptp.tile([P, 2 * NX], F32, name=f"pt{t}", tag="pt") for t in tls
                ]
                for kk in range(KK):
                    g, kkl = kk // 2, kk % 2
                    xg4 = xmg[g][:].rearrange(
                        "p (kkl i n) -> p kkl i n", i=2, n=2 * NX
                    )
                    rhs = xg4[:, kkl]
                    for i, t in enumerate(tls):
                        mt4 = mts[t].rearrange("p (kk j i) -> p kk j i", i=2, j=P)
                        nc.tensor.matmul(
                            out=pts[i][:], lhsT=mt4[:, kk], rhs=rhs,
                            start=(kk == 0), stop=(kk == KK - 1),
                            perf_mode=mybir.MatmulPerfMode.DoubleRowSwInterleave,
                        )
                ob = outp.tile([P, TG * D], F32, tag="ob")
                for i, t in enumerate(tls):
                    pt = pts[i]
                    # s = hi + lo/64 (ACT rescales lo out of PSUM, DVE adds)
                    s1 = outp.tile([P, NX], F32, tag="s1")
                    nc.scalar.activation(
                        out=s1[:], in_=pt[:, NX : 2 * NX],
                        func=mybir.ActivationFunctionType.Copy, scale=1.0 / LO_SCALE,
                    )
                    s2 = outp.tile([P, NX], F32, tag="s2")
                    nc.vector.tensor_add(out=s2[:], in0=s1[:], in1=pt[:, 0:NX])
                    rmax = outp.tile([P, 1], F32, tag="rmax")
                    nc.vector.tensor_scalar_max(
                        out=rmax[:], in0=s2[:, D : D + 1], scalar1=1e-37
                    )
                    rinv = outp.tile([P, 1], F32, tag="rinv")
                    nc.vector.reciprocal(out=rinv[:], in_=rmax[:])
                    nc.vector.tensor_scalar_mul(
                        out=ob[:, i * D : (i + 1) * D], in0=s2[:, 0:D],
                        scalar1=rinv[:, 0:1],
                    )
                nc.sync.dma_start(out=out[tg], in_=ob[:])

    nc.compile()
    return nc


def prep_inputs(spot_emb, W, b, v, gene_ids, spot_ids, T):
    """Host marshaling: shared bf16/f32 operands + per-core fp8 mask slabs."""
    emb = np.ascontiguousarray(np.asarray(spot_emb, dtype=np.float32))
    W = np.asarray(W, dtype=np.float32)
    b = np.asarray(b, dtype=np.float32)
    v = np.asarray(v, dtype=np.float32)
    gene_ids = np.asarray(gene_ids).astype(np.int64)
    spot_ids = np.asarray(spot_ids).astype(np.int64)

    bf = ml_dtypes.bfloat16
    shared = {
        "embc": np.ascontiguousarray(
            emb.reshape(KCH, P, D).transpose(1, 0, 2).reshape(P, KCH * D).astype(bf)
        ),
        "embT": np.ascontiguousarray(emb.T.astype(bf)),
        "wt": np.ascontiguousarray(W.T.astype(bf)),
        "bb": np.ascontiguousarray(b.reshape(D, 1)),
        "vv": np.ascontiguousarray(v.reshape(D, 1).astype(bf)),
    }

    # Dense 0/1 occupancy mask (set semantics: duplicate edges collapse),
    # built directly in the per-core padded layout: core c's genes live at
    # rows [c*T*P, c*T*P + G_PER); rows above G_PER stay zero padding.
    g_pad = T * P
    M = np.zeros((N_CORES * g_pad, N_SPOTS), dtype=bool)
    pad_rows = (gene_ids // G_PER) * g_pad + (gene_ids % G_PER)
    M[pad_rows, spot_ids] = True
    # [c, t*128+g, kk*256 + i*128 + p] -> [c, t, p, kk, jrev, i] where the
    # lhsT free layout per kk is pair-interleaved with gene columns reversed
    # (DoubleRowSwInterleave): flat index 2j+i holds ktile i, gene 127-j
    Mbt = M.reshape(N_CORES, T, P, KK, 2, P)[:, :, ::-1].transpose(0, 1, 5, 3, 2, 4)
    # value 1.0 in fp8e4m3 is byte 0x38
    Mf8 = np.ascontiguousarray(
        (np.ascontiguousarray(Mbt).astype(np.uint8) * 0x38)
        .view(ml_dtypes.float8_e4m3)
        .reshape(N_CORES, T // 2, 2, P, KCH * P)
        .transpose(0, 1, 3, 2, 4)
    ).reshape(N_CORES, T // 2, P, 2 * KCH * P)
    return [{"maskf": Mf8[c], **shared} for c in range(N_CORES)]


_NC_CACHE = {}


def run(spot_emb, W, b, v, gene_ids, spot_ids, trace=False, **hw_kwargs):
    T = (G_PER + P - 1) // P  # 20
    if T not in _NC_CACHE:
        nc = build_nc(T)
        nc.m = get_hw_module(nc.m)
        _NC_CACHE[T] = nc
    nc = _NC_CACHE[T]
    in_maps = prep_inputs(spot_emb, W, b, v, gene_ids, spot_ids, T)
    res = run_bass_kernel_spmd(
        nc, in_maps, core_ids=list(range(N_CORES)), trace=trace, **hw_kwargs
    )
    outs = [
        np.ascontiguousarray(
            np.asarray(res.results[c]["out"], dtype=np.float32)
            .reshape(T // TG, P, TG, D)
            .transpose(0, 2, 1, 3)
        ).reshape(T * P, D)[:G_PER]
        for c in range(N_CORES)
    ]
    full = np.concatenate(outs, axis=0)
    return full, res


def kernel(spot_emb, W, b, v, gene_ids, spot_ids, n_genes):
    n_genes = int(n_genes)
    assert n_genes == N_GENES, f"kernel hardcodes n_genes={N_GENES}, got {n_genes}"
    full, _ = run(spot_emb, W, b, v, gene_ids, spot_ids, trace=False)
    return full
